# revision 1
# baseline (speedup 1.0000x reference)
"""Multi-modality double-value attention on 8 TRN2 NeuronCores.

Sharding: data-parallel over batch (16 items -> 2 per core). Each core runs
the full attention block for its 2 items; weights are replicated. No
collectives. Host pre-transposes x to x^T and casts inputs to bf16; compute
is bf16 with fp32 PSUM accumulation; output is fp32.
"""

import numpy as np
import ml_dtypes

B, N, C = 16, 906, 768
H = 12
D = 64
M1 = 513
N_CORES = 8
BPC = B // N_CORES          # batch items per core
KC = C // 128               # 6 contraction chunks over C
NPAIR = H // 2              # 6 head pairs
NCH = (N + 127) // 128      # 8 key/token chunks over N
KCH = [(i * 128, min(128, N - i * 128)) for i in range(NCH)]
QP = [(0, 512), (512, N - 512)]      # column passes over N
CPASS = [(0, 512), (512, C - 512)]   # column passes over C
SCALE = D ** -0.5
PW = 194  # per-head-pair value block: [V_e(64) | 1 | 1 | 1 | 0*63 | V_o(64)]

TRACE = False          # set by test.py to capture a HW profile
DEBUG_DUMP = False     # add intermediate DRAM outputs (denominators, recips, oT)
LAST_RESULTS = None    # BassKernelResults of the most recent run

_BUILT = None


def _install_trace_shim():
    """The image's antenv lacks axon_hooks; recreate it so trace=True works."""
    import sys, types
    if "antenv.axon_hooks" in sys.modules:
        return
    mod = types.ModuleType("antenv.axon_hooks")
    mod._hook = None
    mod.set_axon_ntff_profile_hook = lambda h: setattr(mod, "_hook", h)
    mod.get_axon_ntff_profile_hook = lambda: mod._hook
    sys.modules["antenv.axon_hooks"] = mod
    import antenv
    antenv.axon_hooks = mod
    from trn_agent_boot.trn_boot import _ntff_profile_via_ctypes
    mod.set_axon_ntff_profile_hook(_ntff_profile_via_ctypes("/opt/axon/libaxon_pjrt.so"))


def _build():
    import concourse.tile as tile
    from concourse import bacc, mybir

    BF = mybir.dt.bfloat16
    F32 = mybir.dt.float32
    AF = mybir.ActivationFunctionType

    nc = bacc.Bacc("TRN2", target_bir_lowering=False, debug=False, num_devices=N_CORES)

    xT_d = nc.dram_tensor("xT", [BPC, C, N], BF, kind="ExternalInput").ap()
    w_d = {
        wn: nc.dram_tensor(wn, [C, C], BF, kind="ExternalInput").ap()
        for wn in ("wq", "wk", "wv", "wvc", "wp")
    }
    bias_d = nc.dram_tensor("bias", [128, C], F32, kind="ExternalInput").ap()
    out_d = nc.dram_tensor("out", [BPC, N, C], F32, kind="ExternalOutput").ap()
    if DEBUG_DUMP:
        dbg_rc = nc.dram_tensor("dbg_rc", [BPC, H, N], F32, kind="ExternalOutput").ap()
        dbg_ot = nc.dram_tensor("dbg_ot", [BPC, NPAIR, 128, N], BF, kind="ExternalOutput").ap()
        dbg_e = nc.dram_tensor("dbg_e", [BPC, 2, NCH, 128, N], BF, kind="ExternalOutput").ap()
        dbg_t1 = nc.dram_tensor("dbg_t1", [BPC, NPAIR, 2, 128, 512], F32, kind="ExternalOutput").ap()
        dbg_v = nc.dram_tensor("dbg_v", [BPC, 3, NCH, 128, NPAIR * PW], BF, kind="ExternalOutput").ap()

    with tile.TileContext(nc) as tc:
        from contextlib import ExitStack
        from concourse import library_config

        with ExitStack() as ctx:
            wpool = ctx.enter_context(tc.tile_pool(name="wpool", bufs=1))
            sb = ctx.enter_context(tc.tile_pool(name="sb", bufs=1))
            ps = ctx.enter_context(tc.tile_pool(name="ps", bufs=1, space="PSUM"))

            # partition_broadcast lives in the gpsimd 'attn' library; the
            # default 'standard' library executes it as garbage on HW
            nc.gpsimd.load_library(library_config.attn)

            # ---- constants: weights + bias ----
            w_sb = {}
            for wn in ("wq", "wk", "wv", "wvc", "wp"):
                tiles = []
                for kc in range(KC):
                    t = wpool.tile([128, C], BF, name=f"{wn}_{kc}", tag=f"{wn}_{kc}")
                    nc.sync.dma_start(t[:], w_d[wn][kc * 128:(kc + 1) * 128, :])
                    tiles.append(t)
                w_sb[wn] = tiles
            bias_sb = wpool.tile([128, C], F32, name="bias_sb", tag="bias_sb")
            nc.sync.dma_start(bias_sb[:], bias_d[:])

            # ---- x^T tiles, both items prefetched ----
            xT = {}
            for it in range(BPC):
                for kc in range(KC):
                    t = sb.tile([128, N], BF, name=f"xT_{it}_{kc}", tag="xT", bufs=6)
                    nc.sync.dma_start(t[:], xT_d[it, kc * 128:(kc + 1) * 128, :])
                    xT[(it, kc)] = t

            for it in range(BPC):
                # ============ projections ============
                qT, kTh = [], []
                for t_ in range(NPAIR):
                    dst = sb.tile([128, N], BF, name=f"qT_{it}_{t_}",
                                  tag="qT", bufs=NPAIR + 1)
                    for (qs, qw) in QP:
                        pp = ps.tile([128, 512], F32, name="pp", tag="ps_mm", bufs=4)
                        with tc.tile_critical():
                            for kc in range(KC):
                                nc.tensor.matmul(
                                    pp[:, 0:qw],
                                    lhsT=w_sb["wq"][kc][:, t_ * 128:(t_ + 1) * 128],
                                    rhs=xT[(it, kc)][:, qs:qs + qw],
                                    start=(kc == 0), stop=(kc == KC - 1),
                                )
                        nc.scalar.copy(dst[:, qs:qs + qw], pp[:, 0:qw])
                    qT.append(dst)
                    # k^T per head, zero-padded to 128 partitions so S^T runs as a
                    # plain K=128 matmul (no PE row tiling -- T8 tile corrupts on HW)
                    ke = sb.tile([128, N], BF, name=f"kTh_{it}_{2*t_}", tag="kT", bufs=2 * NPAIR + 1)
                    ko = sb.tile([128, N], BF, name=f"kTh_{it}_{2*t_+1}", tag="kT", bufs=2 * NPAIR + 1)
                    nc.vector.memset(ke[64:128, :], 0.0)
                    nc.vector.memset(ko[0:64, :], 0.0)
                    for (qs, qw) in QP:
                        pp = ps.tile([128, 512], F32, name="pp", tag="ps_mm", bufs=4)
                        with tc.tile_critical():
                            for kc in range(KC):
                                nc.tensor.matmul(
                                    pp[:, 0:qw],
                                    lhsT=w_sb["wk"][kc][:, t_ * 128:(t_ + 1) * 128],
                                    rhs=xT[(it, kc)][:, qs:qs + qw],
                                    start=(kc == 0), stop=(kc == KC - 1),
                                )
                        nc.scalar.copy(ke[0:64, qs:qs + qw], pp[0:64, 0:qw])
                        nc.scalar.copy(ko[64:128, qs:qs + qw], pp[64:128, 0:qw])
                    kTh.append(ke)
                    kTh.append(ko)

                v_sb, vc_sb = [], []
                for c, (ts, tsz) in enumerate(KCH):
                    for dst_list, wn, tg in ((v_sb, "wv", "v"), (vc_sb, "wvc", "vc")):
                        dst = sb.tile([128, NPAIR * PW], BF, name=f"{tg}_{it}_{c}",
                                      tag=tg, bufs=NCH + 1)
                        if tsz < 128:
                            # stationary loads may read all 128 partitions; keep
                            # the unwritten tail finite
                            nc.vector.memset(dst[:, :], 0.0)
                        dvw = dst[0:tsz, :].rearrange("p (g c) -> p g c", c=PW)
                        for (cs, cw) in CPASS:
                            pp = ps.tile([128, 512], F32, name="pp", tag="ps_mm", bufs=4)
                            with tc.tile_critical():
                                for kc in range(KC):
                                    nc.tensor.matmul(
                                        pp[0:tsz, 0:cw],
                                        lhsT=xT[(it, kc)][:, ts:ts + tsz],
                                        rhs=w_sb[wn][kc][:, cs:cs + cw],
                                        start=(kc == 0), stop=(kc == KC - 1),
                                    )
                            g0, gn = (0, 4) if cs == 0 else (4, 2)
                            src = pp[0:tsz, 0:cw].rearrange("p (g r d) -> p g r d", r=2, d=D)
                            nc.scalar.copy(dvw[:, g0:g0 + gn, 0:D], src[:, :, 0, :])
                            nc.scalar.copy(dvw[:, g0:g0 + gn, 130:194], src[:, :, 1, :])
                        nc.vector.memset(dvw[:, :, 64:67], 1.0)
                        nc.vector.memset(dvw[:, :, 67:130], 0.0)
                        dst_list.append(dst)

                # mixed tiles for the key chunk straddling M1 (chunk 4: keys 512..639)
                amix = sb.tile([128, NPAIR * PW], BF, name=f"amix_{it}", tag="amix", bufs=BPC)
                vmix = sb.tile([128, NPAIR * PW], BF, name=f"vmix_{it}", tag="vmix", bufs=BPC)
                nc.vector.tensor_copy(amix[:, :], vc_sb[4][:, :])
                nc.vector.tensor_copy(amix[0:1, :], v_sb[4][0:1, :])
                nc.vector.tensor_copy(vmix[:, :], v_sb[4][:, :])
                nc.vector.tensor_copy(vmix[0:1, :], vc_sb[4][0:1, :])
                if DEBUG_DUMP:
                    for c_ in range(NCH):
                        nc.sync.dma_start(dbg_v[it, 0, c_], v_sb[c_][:, :])
                        nc.sync.dma_start(dbg_v[it, 1, c_], vc_sb[c_][:, :])
                    nc.sync.dma_start(dbg_v[it, 2, 0], amix[:, :])
                    nc.sync.dma_start(dbg_v[it, 2, 1], vmix[:, :])

                # ============ attention, one head pair at a time ============
                oT = []
                for p in range(NPAIR):
                    # S^T = scores transposed (keys on partitions), then exp
                    exps = {}
                    for c, (ks, ksz) in enumerate(KCH):
                        for par in range(2):
                            # exp split into 4-byte-aligned tiles: eA = q[0,512),
                            # eR = q=512, eB = q[513,906)
                            eA = sb.tile([128, 512], BF, name="eA", tag="expA", bufs=17)
                            eB = sb.tile([128, 394], BF, name="eB", tag="expB", bufs=17)
                            eR = sb.tile([128, 2], BF, name="eR", tag="expR", bufs=17)
                            pst = ps.tile([128, 512], F32, name="pst", tag="ps_s", bufs=4)
                            nc.tensor.matmul(pst[0:ksz, 0:512],
                                             lhsT=kTh[2 * p + par][:, ks:ks + ksz],
                                             rhs=qT[p][:, 0:512], start=True, stop=True)
                            nc.scalar.activation(eA[0:ksz, :], pst[0:ksz, 0:512],
                                                 AF.Exp, scale=SCALE)
                            pst2 = ps.tile([128, 512], F32, name="pst2", tag="ps_s", bufs=4)
                            nc.tensor.matmul(pst2[0:ksz, 0:394],
                                             lhsT=kTh[2 * p + par][:, ks:ks + ksz],
                                             rhs=qT[p][:, 512:906], start=True, stop=True)
                            nc.scalar.activation(eR[0:ksz, 0:1], pst2[0:ksz, 0:1],
                                                 AF.Exp, scale=SCALE)
                            nc.scalar.activation(eB[0:ksz, 0:393], pst2[0:ksz, 1:394],
                                                 AF.Exp, scale=SCALE)
                            exps[(c, par)] = (eA, eR, eB)

                    ot = sb.tile([128, N], BF, name=f"oT_{it}_{p}", tag="oT", bufs=NPAIR + 1)
                    for par in range(2):
                        # three accumulators, each a single bank holding a single
                        # accumulation group: q[0,512) a-mod, q=512 a-mod, q[513,906) v-mod
                        t1 = ps.tile([128, 512], F32, name="t1", tag="ps_mm", bufs=4)
                        t2 = ps.tile([128, 512], F32, name="t2", tag="ps_mm", bufs=4)
                        if par == 0:
                            o1, o2 = t1[0:65, 0:512], t2[0:65, 0:393]
                            mrows = slice(0, 65)
                            csl = slice(p * PW, p * PW + 65)          # [V_even | 1]
                            drow, orows = 64, slice(0, 64)
                        else:
                            o1, o2 = t1[:, 0:512], t2[:, 0:393]
                            mrows = slice(0, 128)
                            csl = slice(p * PW + 66, p * PW + PW)     # [1 | 0*63 | V_odd]
                            drow, orows = 0, slice(64, 128)
                        # modality-a queries: q in [0,512) -> o1
                        with tc.tile_critical():
                            for c, (ks, ksz) in enumerate(KCH):
                                va = amix if c == 4 else (v_sb[c] if c < 4 else vc_sb[c])
                                nc.tensor.matmul(o1, lhsT=va[0:ksz, csl],
                                                 rhs=exps[(c, par)][0][0:ksz, 0:512],
                                                 start=(c == 0), stop=(c == NCH - 1),
                                                 tile_position=(0, 0))
                        # a-modality q=512 column: 8 independent single-matmul
                        # writes (start&stop each), accumulated on the DVE --
                        # avoids a long-lived 1-wide PSUM accumulation group
                        racc = sb.tile([128, 4], F32, name="racc", tag="racc", bufs=2)
                        nc.vector.memset(racc[:, 0:1], 0.0)
                        for c, (ks, ksz) in enumerate(KCH):
                            va = amix if c == 4 else (v_sb[c] if c < 4 else vc_sb[c])
                            rc1 = ps.tile([128, 512], F32, name="rc1", tag="ps_s", bufs=4)
                            nc.tensor.matmul(rc1[mrows, 0:1], lhsT=va[0:ksz, csl],
                                             rhs=exps[(c, par)][1][0:ksz, 0:1],
                                             start=True, stop=True, tile_position=(0, 0))
                            nc.vector.tensor_add(racc[mrows, 0:1], racc[mrows, 0:1],
                                                 rc1[mrows, 0:1])
                        # modality-v queries: q in [513,906) -> o2
                        with tc.tile_critical():
                            for c, (ks, ksz) in enumerate(KCH):
                                vv = vmix if c == 4 else (vc_sb[c] if c < 4 else v_sb[c])
                                nc.tensor.matmul(o2, lhsT=vv[0:ksz, csl],
                                                 rhs=exps[(c, par)][2][0:ksz, 0:393],
                                                 start=(c == 0), stop=(c == NCH - 1),
                                                 tile_position=(0, 0))
                        # softmax division: denominators sit in row `drow`
                        bcs = sb.tile([128, N], F32, name="bcs", tag="bc", bufs=2)
                        nc.vector.reciprocal(bcs[drow:drow + 1, 0:512], t1[drow:drow + 1, 0:512])
                        nc.vector.reciprocal(bcs[drow:drow + 1, 512:513], racc[drow:drow + 1, 0:1])
                        nc.vector.reciprocal(bcs[drow:drow + 1, 513:906], t2[drow:drow + 1, 0:393])
                        if drow != 0:
                            # hw partition_broadcast reads physical partition 0;
                            # relocate the reciprocal row there first
                            nc.sync.dma_start(bcs[0:1, 0:906], bcs[drow:drow + 1, 0:906])
                        bc2 = sb.tile([128, N], F32, name="bc2", tag="bc2", bufs=2)
                        nc.gpsimd.partition_broadcast(bc2[:, 0:906], bcs[0:1, 0:906])
                        nc.vector.tensor_mul(ot[orows, 0:512], t1[orows, 0:512], bc2[orows, 0:512])
                        nc.vector.tensor_mul(ot[orows, 512:513], racc[orows, 0:1], bc2[orows, 512:513])
                        nc.vector.tensor_mul(ot[orows, 513:906], t2[orows, 0:393], bc2[orows, 513:906])
                        if DEBUG_DUMP:
                            h = 2 * p + par
                            nc.sync.dma_start(dbg_rc[it, h, 0:512], bc[drow:drow + 1, 0:512])
                            nc.sync.dma_start(dbg_rc[it, h, 512:906], bc[drow:drow + 1, 512:906])
                            t1c = sb.tile([128, 512], F32, name="t1c", tag="t1c", bufs=2)
                            nc.vector.tensor_copy(t1c[:, :], t1[:, :])
                            nc.sync.dma_start(dbg_t1[it, p, par], t1c[:, :])
                    if DEBUG_DUMP:
                        nc.sync.dma_start(dbg_ot[it, p], ot[:, :])
                        if p == 1:
                            for par_ in range(2):
                                for c_ in range(NCH):
                                    nc.sync.dma_start(dbg_e[it, par_, c_, :, 0:512], exps[(c_, par_)][0][:, :])
                    oT.append(ot)

                # ============ output projection + bias ============
                for c, (ts, tsz) in enumerate(KCH):
                    for (cs, cw) in CPASS:
                        pp = ps.tile([128, 512], F32, name="pp", tag="ps_mm", bufs=4)
                        with tc.tile_critical():
                            for kp in range(NPAIR):
                                nc.tensor.matmul(
                                    pp[0:tsz, 0:cw],
                                    lhsT=oT[kp][:, ts:ts + tsz],
                                    rhs=w_sb["wp"][kp][:, cs:cs + cw],
                                    start=(kp == 0), stop=(kp == NPAIR - 1),
                                )
                        ob = sb.tile([128, 512], F32, name="ob", tag="ob", bufs=2)
                        nc.vector.tensor_add(ob[0:tsz, 0:cw], pp[0:tsz, 0:cw],
                                             bias_sb[0:tsz, cs:cs + cw])
                        nc.sync.dma_start(out_d[it, ts:ts + tsz, cs:cs + cw], ob[0:tsz, 0:cw])

    nc.compile()
    return nc


def _get_built():
    global _BUILT
    if _BUILT is None:
        _BUILT = _build()
    return _BUILT


def kernel(x, Wq, Wk, Wv, Wvc, Wp, bp):
    global LAST_RESULTS
    from concourse.bass_utils import run_bass_kernel_spmd

    x = np.asarray(x, dtype=np.float32)
    bf = ml_dtypes.bfloat16
    xT = np.ascontiguousarray(x.transpose(0, 2, 1)).astype(bf)      # (B, C, N)
    ws = {
        "wq": np.asarray(Wq, dtype=np.float32).astype(bf),
        "wk": np.asarray(Wk, dtype=np.float32).astype(bf),
        "wv": np.asarray(Wv, dtype=np.float32).astype(bf),
        "wvc": np.asarray(Wvc, dtype=np.float32).astype(bf),
        "wp": np.asarray(Wp, dtype=np.float32).astype(bf),
    }
    bias = np.ascontiguousarray(
        np.broadcast_to(np.asarray(bp, dtype=np.float32), (128, C))
    )

    if TRACE:
        _install_trace_shim()

    nc = _get_built()
    in_maps = []
    for i in range(N_CORES):
        m = {"xT": np.ascontiguousarray(xT[i * BPC:(i + 1) * BPC]), "bias": bias}
        m.update(ws)
        in_maps.append(m)

    res = run_bass_kernel_spmd(nc, in_maps, list(range(N_CORES)), trace=TRACE,
                               stitch_traces=False)
    LAST_RESULTS = res
    out = np.concatenate([res.results[i]["out"] for i in range(N_CORES)], axis=0)
    return out



# revision 29
# speedup vs baseline: 2.5870x; 2.5870x over previous
"""Multi-modality double-value attention on 8 TRN2 NeuronCores.

Sharding: data-parallel over batch (16 items -> 2 per core). Each core runs
the full attention block for its 2 items; weights are replicated. No
collectives. Host pre-transposes x to x^T and casts inputs to bf16; compute
is bf16 with fp32 PSUM accumulation; output is fp32.

v2 design notes (vs v1 baseline):
- No tile_critical sections (they chain-serialize with engine drains).
- S^T lands in one 2-bank PSUM tile -> single Exp call per (head, chunk).
- q=512 straddle column is an in-PSUM 1-wide accumulation group.
- Softmax divide: denom row -> scalar copies -> DVE reciprocal_approx_fast
  -> partition-broadcast DMA (scalar queue) -> DVE muls. No gpsimd.
- Value/k^T tiles persistent across items; pad zeros memset once.
- PSUM evacuations on DVE; ScalarE kept for exp + denom row copies.
"""

import numpy as np
import ml_dtypes

B, N, C = 16, 906, 768
H = 12
D = 64
M1 = 513
N_CORES = 8
BPC = B // N_CORES          # batch items per core
KC = C // 128               # 6 contraction chunks over C
NPAIR = H // 2              # 6 head pairs
NCH = (N + 127) // 128      # 8 key/token chunks over N
KCH = [(i * 128, min(128, N - i * 128)) for i in range(NCH)]
QP = [(0, 512), (512, N - 512)]      # column passes over N
CPASS = [(0, 512), (512, C - 512)]   # column passes over C
SCALE = D ** -0.5
PW = 194  # per-head-pair value block: [V_e(64) | 1 1 1 | 0*63 | V_o(64)]

TRACE = False          # set by test.py to capture a HW profile
DEBUG_DUMP = False
LAST_RESULTS = None    # BassKernelResults of the most recent run

_BUILT = None


def _install_trace_shim():
    """The image's antenv lacks axon_hooks; recreate it so trace=True works."""
    import sys, types
    if "antenv.axon_hooks" in sys.modules:
        return
    mod = types.ModuleType("antenv.axon_hooks")
    mod._hook = None
    mod.set_axon_ntff_profile_hook = lambda h: setattr(mod, "_hook", h)
    mod.get_axon_ntff_profile_hook = lambda: mod._hook
    sys.modules["antenv.axon_hooks"] = mod
    import antenv
    antenv.axon_hooks = mod
    from trn_agent_boot.trn_boot import _ntff_profile_via_ctypes
    mod.set_axon_ntff_profile_hook(_ntff_profile_via_ctypes("/opt/axon/libaxon_pjrt.so"))


def _build():
    import concourse.tile as tile
    from concourse import bacc, mybir

    BF = mybir.dt.bfloat16
    F32 = mybir.dt.float32
    AF = mybir.ActivationFunctionType

    nc = bacc.Bacc("TRN2", target_bir_lowering=False, debug=False, num_devices=N_CORES)

    xT_d = nc.dram_tensor("xT", [BPC, C, N], BF, kind="ExternalInput").ap()
    w_d = {
        wn: nc.dram_tensor(wn, [C, C], BF, kind="ExternalInput").ap()
        for wn in ("wq", "wk", "wv", "wvc", "wp")
    }
    bias_d = nc.dram_tensor("bias", [128, C], F32, kind="ExternalInput").ap()
    out_d = nc.dram_tensor("out", [BPC, N, C], F32, kind="ExternalOutput").ap()
    if DEBUG_DUMP:
        dbg_drow = nc.dram_tensor("dbg_drow", [BPC, NPAIR, 2, 908], F32, kind="ExternalOutput").ap()
        dbg_drec = nc.dram_tensor("dbg_drec", [BPC, NPAIR, 2, 908], F32, kind="ExternalOutput").ap()
        dbg_bc = nc.dram_tensor("dbg_bc", [BPC, NPAIR, 2, 4, 908], F32, kind="ExternalOutput").ap()
        dbg_ot = nc.dram_tensor("dbg_ot", [BPC, NPAIR, 128, N], BF, kind="ExternalOutput").ap()
        dbg_e = nc.dram_tensor("dbg_e", [BPC, 2, NCH, 128, N], BF, kind="ExternalOutput").ap()

    with tile.TileContext(nc) as tc:
        from contextlib import ExitStack

        with ExitStack() as ctx:
            from concourse import library_config

            wpool = ctx.enter_context(tc.tile_pool(name="wpool", bufs=1))
            sb = ctx.enter_context(tc.tile_pool(name="sb", bufs=1))
            ps = ctx.enter_context(tc.tile_pool(name="ps", bufs=1, space="PSUM"))

            # partition_broadcast lives in the gpsimd 'attn' library
            nc.gpsimd.load_library(library_config.attn)

            # ---- constants: weights + bias ----
            w_sb = {}
            for wn in ("wq", "wk", "wv", "wvc", "wp"):
                tiles = []
                for kc in range(KC):
                    t = wpool.tile([128, C], BF, name=f"{wn}_{kc}", tag=f"{wn}_{kc}")
                    nc.sync.dma_start(t[:], w_d[wn][kc * 128:(kc + 1) * 128, :])
                    tiles.append(t)
                w_sb[wn] = tiles
            bias_sb = wpool.tile([128, C], F32, name="bias_sb", tag="bias_sb")
            nc.sync.dma_start(bias_sb[:], bias_d[:])
            # zeros row used as a K=1 dummy matmul weight to pre-clear the
            # shared tB PSUM bank (opens one has_written domain for both
            # accumulation groups living in that bank)
            zrow = wpool.tile([128, 128], BF, name="zrow", tag="zrow")
            nc.vector.memset(zrow[:, :], 0.0)

            # ---- persistent attention tiles (written every item; pads
            #      memset once) ----
            # k^T per head, zero-padded to 128 partitions so S^T runs as a
            # plain K=128 matmul
            kTh = []
            for h in range(2 * NPAIR):
                t = wpool.tile([128, N], BF, name=f"kTh_{h}", tag=f"kTh_{h}")
                if h % 2 == 0:
                    nc.vector.memset(t[64:128, :], 0.0)
                else:
                    nc.vector.memset(t[0:64, :], 0.0)
                kTh.append(t)
            # value blocks: per chunk, 6 pairs * [V_e(64) | 1 1 1 | 0*63 | V_o(64)]
            v_sb, vc_sb = [], []
            for c, (ts, tsz) in enumerate(KCH):
                for dst_list, tg in ((v_sb, "v"), (vc_sb, "vc")):
                    t = wpool.tile([128, NPAIR * PW], BF, name=f"{tg}_{c}", tag=f"{tg}_{c}")
                    tv = t[:].rearrange("p (g c) -> p g c", c=PW)
                    # zero everything (incl. pad rows of the short last chunk --
                    # stationary loads may read all 128 partitions), then paint
                    # the ones columns
                    nc.vector.memset(t[:, :], 0.0)
                    nc.vector.memset(tv[:, :, 64:67], 1.0)
                    dst_list.append(t)
            amix = wpool.tile([128, NPAIR * PW], BF, name="amix", tag="amix")
            vmix = wpool.tile([128, NPAIR * PW], BF, name="vmix", tag="vmix")

            # ---- x^T tiles, both items prefetched (keeps item-1 loads ahead
            #      of item-0 output stores on the sync DMA queue) ----
            xT_all = {}
            for it in range(BPC):
                for kc in range(KC):
                    t = sb.tile([128, N], BF, name=f"xT_{it}_{kc}", tag="xT",
                                bufs=BPC * KC)
                    nc.sync.dma_start(t[:], xT_d[it, kc * 128:(kc + 1) * 128, :])
                    xT_all[(it, kc)] = t

            for it in range(BPC):
                xT = [xT_all[(it, kc)] for kc in range(KC)]

                # ============ projections ============
                # q: per pair, both QP passes into one 2-bank PSUM tile
                qT = []
                for t_ in range(NPAIR):
                    pp = ps.tile([128, 1024], F32, name="pst", tag="pst", bufs=2)
                    for (qs, qw) in QP:
                        for kc in range(KC):
                            nc.tensor.matmul(
                                pp[:, qs:qs + qw],
                                lhsT=w_sb["wq"][kc][:, t_ * 128:(t_ + 1) * 128],
                                rhs=xT[kc][:, qs:qs + qw],
                                start=(kc == 0), stop=(kc == KC - 1),
                            )
                    dst = sb.tile([128, N], BF, name=f"qT_{it}_{t_}",
                                  tag="qT", bufs=NPAIR + 1)
                    nc.vector.tensor_copy(dst[:, 0:N], pp[:, 0:N])
                    qT.append(dst)
                # k: per pair, evac split into the even/odd padded kTh tiles
                for t_ in range(NPAIR):
                    pp = ps.tile([128, 1024], F32, name="pst", tag="pst", bufs=2)
                    for (qs, qw) in QP:
                        for kc in range(KC):
                            nc.tensor.matmul(
                                pp[:, qs:qs + qw],
                                lhsT=w_sb["wk"][kc][:, t_ * 128:(t_ + 1) * 128],
                                rhs=xT[kc][:, qs:qs + qw],
                                start=(kc == 0), stop=(kc == KC - 1),
                            )
                    nc.vector.tensor_copy(kTh[2 * t_][0:64, 0:N], pp[0:64, 0:N])
                    nc.vector.tensor_copy(kTh[2 * t_ + 1][64:128, 0:N], pp[64:128, 0:N])
                # v / vc: per token chunk, both CPASS into one 2-bank tile
                for c, (ts, tsz) in enumerate(KCH):
                    for dst, wn in ((v_sb[c], "wv"), (vc_sb[c], "wvc")):
                        pp = ps.tile([128, 1024], F32, name="pst", tag="pst", bufs=2)
                        for (cs, cw) in CPASS:
                            for kc in range(KC):
                                nc.tensor.matmul(
                                    pp[0:tsz, cs:cs + cw],
                                    lhsT=xT[kc][:, ts:ts + tsz],
                                    rhs=w_sb[wn][kc][:, cs:cs + cw],
                                    start=(kc == 0), stop=(kc == KC - 1),
                                )
                        dvw = dst[0:tsz, :].rearrange("p (g c) -> p g c", c=PW)
                        src = pp[0:tsz, 0:C].rearrange("p (g r d) -> p g r d", r=2, d=D)
                        nc.vector.tensor_copy(dvw[:, :, 0:D], src[:, :, 0, :])
                        nc.vector.tensor_copy(dvw[:, :, 130:194], src[:, :, 1, :])
                # mixed tiles for the key chunk straddling M1 (keys 512..639)
                nc.vector.tensor_copy(amix[:, :], vc_sb[4][:, :])
                nc.vector.tensor_copy(amix[0:1, :], v_sb[4][0:1, :])
                nc.vector.tensor_copy(vmix[:, :], v_sb[4][:, :])
                nc.vector.tensor_copy(vmix[0:1, :], vc_sb[4][0:1, :])

                # ============ attention, one head pair at a time ============
                oT = []
                for p in range(NPAIR):
                    exps = {}
                    for par in range(2):
                        for c, (ks, ksz) in enumerate(KCH):
                            pst = ps.tile([128, 1024], F32, name="pst", tag="pst", bufs=2)
                            nc.tensor.matmul(pst[0:ksz, 0:512],
                                             lhsT=kTh[2 * p + par][:, ks:ks + ksz],
                                             rhs=qT[p][:, 0:512], start=True, stop=True)
                            nc.tensor.matmul(pst[0:ksz, 512:906],
                                             lhsT=kTh[2 * p + par][:, ks:ks + ksz],
                                             rhs=qT[p][:, 512:906], start=True, stop=True)
                            eF = sb.tile([128, N], BF, name="eF", tag="eF", bufs=11)
                            nc.scalar.activation(eF[0:ksz, 0:N], pst[0:ksz, 0:N],
                                                 AF.Exp, scale=SCALE)
                            if DEBUG_DUMP and p == 0:
                                nc.sync.dma_start(dbg_e[it, par, c], eF[:, :])
                            exps[(c, par)] = eF

                    ot = sb.tile([128, N], BF, name=f"oT_{it}_{p}", tag="oT",
                                 bufs=NPAIR + 2)
                    for par in range(2):
                        if par == 0:
                            mrows = slice(0, 65)
                            csl = slice(p * PW, p * PW + 65)          # [V_e | 1]
                            drow, orows = 64, slice(0, 64)
                        else:
                            mrows = slice(0, 128)
                            csl = slice(p * PW + 66, p * PW + PW)     # [1 | 0*63 | V_o]
                            drow, orows = 0, slice(64, 128)
                        # three interleaved accumulation groups:
                        #   t1 [*,512]  a-mod queries 0..511
                        #   tB col 0    a-mod query 512 (1-wide)
                        #   tB 1:395    v-mod queries 512..905 (col 1 = q512
                        #               computed with wrong values, overwritten)
                        t1 = ps.tile([128, 512], F32, name="t1", tag="pv", bufs=4)
                        tB = ps.tile([128, 512], F32, name="tB", tag="pv", bufs=4)
                        # tB holds two accumulation groups in one bank (col 0:
                        # a-mod q512; cols 1:395: v-mod q512..905). A start=True
                        # in either group would clear the WHOLE bank's
                        # has_written bits, clobbering the other group -- so a
                        # zero K=1 dummy matmul opens the bank once and both
                        # groups accumulate with start=False. Groups run as
                        # contiguous 8-MM passes (interleaving open groups
                        # chunk-wise corrupted PSUM on HW).
                        def vA(c):
                            return amix if c == 4 else (v_sb[c] if c < 4 else vc_sb[c])

                        def vV(c):
                            return vmix if c == 4 else (vc_sb[c] if c < 4 else v_sb[c])

                        for c, (ks, ksz) in enumerate(KCH):
                            nc.tensor.matmul(t1[mrows, 0:512], lhsT=vA(c)[0:ksz, csl],
                                             rhs=exps[(c, par)][0:ksz, 0:512],
                                             start=(c == 0), stop=(c == NCH - 1))
                        nc.tensor.matmul(tB[0:128, 0:512], lhsT=zrow[0:1, 0:128],
                                         rhs=w_sb["wq"][0][0:1, 0:512],
                                         start=True, stop=False)
                        for c, (ks, ksz) in enumerate(KCH):
                            nc.tensor.matmul(tB[mrows, 0:1], lhsT=vA(c)[0:ksz, csl],
                                             rhs=exps[(c, par)][0:ksz, 512:513],
                                             start=False, stop=(c == NCH - 1))
                        for c, (ks, ksz) in enumerate(KCH):
                            nc.tensor.matmul(tB[mrows, 1:395], lhsT=vV(c)[0:ksz, csl],
                                             rhs=exps[(c, par)][0:ksz, 512:906],
                                             start=False, stop=(c == NCH - 1))
                        # softmax denominators sit in row `drow`; keep all ops
                        # on that same partition, then broadcast via a DRAM
                        # bounce (engines cannot shift partitions; SBUF DMA
                        # rejects zero-step partition APs, DRAM allows them).
                        # d_row layout: [0:512]=a q0..511, [512]=a q512,
                        # [513]=junk, [514:907]=v q513..905
                        d_row = sb.tile([128, 908], F32, name="d_row", tag="d_row", bufs=2)
                        r = slice(drow, drow + 1)
                        nc.vector.tensor_copy(d_row[r, 0:512], t1[r, 0:512])
                        nc.vector.tensor_copy(d_row[r, 512:907], tB[r, 0:395])
                        if drow != 0:
                            # reciprocal_approx_fast and partition_broadcast
                            # both need partition 0; relocate the raw row first
                            nc.sync.dma_start(d_row[0:1, 0:907], d_row[r, 0:907])
                        d_rec = sb.tile([128, 908], F32, name="d_rec", tag="d_rec", bufs=2)
                        nc.vector.reciprocal_approx_fast(d_rec[0:1, 0:907],
                                                         d_row[0:1, 0:907])
                        bc = sb.tile([128, 908], F32, name="bc", tag="bc", bufs=2)
                        nc.gpsimd.partition_broadcast(bc[:, 0:907], d_rec[0:1, 0:907])
                        if DEBUG_DUMP:
                            nc.sync.dma_start(dbg_drow[it, p, par], d_row[0:1, 0:908])
                            nc.sync.dma_start(dbg_drec[it, p, par], d_rec[0:1, 0:908])
                            nc.sync.dma_start(dbg_bc[it, p, par, 0:2], bc[0:2, 0:908])
                            nc.sync.dma_start(dbg_bc[it, p, par, 2:4], bc[64:66, 0:908])
                        nc.vector.tensor_mul(ot[orows, 0:512], t1[orows, 0:512],
                                             bc[orows, 0:512])
                        nc.vector.tensor_mul(ot[orows, 512:906], tB[orows, 1:395],
                                             bc[orows, 513:907])
                        # q512 rewrite must come after the v-mod mul (WAW)
                        nc.vector.tensor_mul(ot[orows, 512:513], tB[orows, 0:1],
                                             bc[orows, 512:513])
                    if DEBUG_DUMP:
                        nc.sync.dma_start(dbg_ot[it, p], ot[:, :])
                    oT.append(ot)

                # ============ output projection + bias ============
                for c, (ts, tsz) in enumerate(KCH):
                    pp = ps.tile([128, 1024], F32, name="pst", tag="pst", bufs=2)
                    for (cs, cw) in CPASS:
                        for kp in range(NPAIR):
                            nc.tensor.matmul(
                                pp[0:tsz, cs:cs + cw],
                                lhsT=oT[kp][:, ts:ts + tsz],
                                rhs=w_sb["wp"][kp][:, cs:cs + cw],
                                start=(kp == 0), stop=(kp == NPAIR - 1),
                            )
                    ob = sb.tile([128, C], F32, name="ob", tag="ob", bufs=2)
                    nc.vector.tensor_add(ob[0:tsz, 0:C], pp[0:tsz, 0:C],
                                         bias_sb[0:tsz, 0:C])
                    nc.sync.dma_start(out_d[it, ts:ts + tsz, 0:C], ob[0:tsz, 0:C])

    nc.compile()
    return nc


def _get_built():
    global _BUILT
    if _BUILT is None:
        _BUILT = _build()
    return _BUILT


def kernel(x, Wq, Wk, Wv, Wvc, Wp, bp):
    global LAST_RESULTS
    from concourse.bass_utils import run_bass_kernel_spmd

    x = np.asarray(x, dtype=np.float32)
    bf = ml_dtypes.bfloat16
    xT = np.ascontiguousarray(x.transpose(0, 2, 1)).astype(bf)      # (B, C, N)
    ws = {
        "wq": np.asarray(Wq, dtype=np.float32).astype(bf),
        "wk": np.asarray(Wk, dtype=np.float32).astype(bf),
        "wv": np.asarray(Wv, dtype=np.float32).astype(bf),
        "wvc": np.asarray(Wvc, dtype=np.float32).astype(bf),
        "wp": np.asarray(Wp, dtype=np.float32).astype(bf),
    }
    bias = np.ascontiguousarray(
        np.broadcast_to(np.asarray(bp, dtype=np.float32), (128, C))
    )

    if TRACE:
        _install_trace_shim()

    nc = _get_built()
    in_maps = []
    for i in range(N_CORES):
        m = {"xT": np.ascontiguousarray(xT[i * BPC:(i + 1) * BPC]), "bias": bias}
        m.update(ws)
        in_maps.append(m)

    res = run_bass_kernel_spmd(nc, in_maps, list(range(N_CORES)), trace=TRACE,
                               stitch_traces=False)
    LAST_RESULTS = res
    out = np.concatenate([res.results[i]["out"] for i in range(N_CORES)], axis=0)
    return out
